# revision 63
# baseline (speedup 1.0000x reference)
"""Trainium2 Bass kernel for nn_AttentionWeight (GAT edge softmax).

out[e,h] = softmax_over_dst_segments(relu(el[src]+er[dst]+ee[etype]))

Math used on device (er cancels out of the softmax):
  exp(relu(x)) = max(exp(x), 1),  exp(x) = el'*ee'*er'   (primes = exp factors)
  max(el'*ee'*er', 1) = er' * max(el'*ee', 1/er') = er' * max(m2, ir)
  out = max(m2, ir) / segment_sum(max(m2, ir))           (er' cancels; ir=1/er')
  Padding slots carry m2 = 0 and ir = 0, so q = max(m2, ir) = 0 there and the
  segment sum needs no degree correction.

Distribution (8 NeuronCores, two SPMD launches):
  Launch A: node-sharded projections (fp16 feat, fp16 matmul). Core s owns
    nodes [12500s, 12500(s+1)) on an out-degree-sorted grid [128 x 98] (the
    sort is a host-side column permutation of featT):
      el' = exp(+logit_l), ir' = exp(-logit_r)  (ACT, fp16 out)
      ee' = exp(contract(edge_emb@W_e, attn_e))       [8 x 8]
    The per-edge el' values are emitted on device: for each node group, its
    el' row is replicated across that node's out-edge slot columns (DVE 4x
    fp16 broadcast copy), so every edge's el' factor leaves the device in
    src-grid slot order (m1s).
  Host: bijectively re-shards the per-edge el' slot values from src-grid to
    dst-grid order (each edge's 8 floats appear exactly once on each side),
    and expands the tiny device-computed ee' [8x8] / ir' [nodes x 8] tables
    into per-slot marshalled inputs. Integer index arrays + pure permutation /
    replication of device-produced floats only -- no float arithmetic.
  Launch B: edge/dst-sharded softmax. Core c owns dst range on an in-degree-
    sorted grid, edges padded into chunk-uniform slot bands (~3-5% padding),
    inputs packed [el | ee | ir] per chunk so each chunk is one contiguous
    load, software-pipelined 4 chunks ahead. Per chunk (fp16, DVE 2x):
    m2 = el*ee; q = max(m2, ir); segment sums via fp16 pair-add + strided
    X-reduce (f32 accumulate); r = 1/s; out = q*r. Contiguous DMA only --
    no indirect gathers.
  Host: scatters padded slots back to original edge order (indexing only).

All floating-point arithmetic happens on device; the host only shards,
permutes, concatenates and builds integer index/count arrays.
"""

import sys

sys.path.insert(0, "/opt/trn_rl_repo")

import numpy as np

import concourse.bacc as bacc
import concourse.mybir as mybir
import concourse.tile as tile
from concourse.bass_utils import run_bass_kernel_spmd

# problem constants (hardcoded per harness contract)
N = 100000
E = 3200000
IN = 256
H = 8
O = 64
F = 64
T = 8
NCORES = 8
P = 128

NS = N // NCORES            # 12500 nodes per shard
NSP = 12544                 # padded to 128*98
G = NSP // P                # 98 groups of 128 nodes

FP = mybir.dt.float32
BF = mybir.dt.float16
I32 = mybir.dt.int32

CH = 8                      # groups per chunk (uniform slot width per chunk)
SLICES = 32                 # node groups per PSUM bank / featT slab

_timings = {}


def _chunk_layout(gw):
    """gw: per-group slot width [G]. Returns list of (g0, g1, wc, colbase)
    with uniform width wc = max(gw[g0:g1]) per chunk, and total columns."""
    chunks = []
    colbase = 0
    g0 = 0
    while g0 < G:
        g1 = min(g0 + CH, G)
        wc = int(max(2, max(gw[g0:g1])))
        wc += wc & 1                       # even width (pair-add reduce)
        chunks.append((g0, g1, wc, colbase))
        colbase += (g1 - g0) * wc
        g0 = g1
    return chunks, colbase


# ---------------------------------------------------------------------------
# Launch A: projections + per-edge el' emission (src grid)
# ---------------------------------------------------------------------------

def _build_launch_a(chunks_a, ka):
    nc = bacc.Bacc("TRN2", target_bir_lowering=False, debug=False,
                   num_devices=NCORES)
    # featT columns are t-major: column g*128 + p holds (sorted) grid node (p, g)
    featT = nc.dram_tensor("featT", [IN, NSP], BF, kind="ExternalInput")
    w_fc = nc.dram_tensor("w_fc", [IN, H * O], FP, kind="ExternalInput")
    attn_lr = nc.dram_tensor("attn_lr", [P, 2 * H * O], FP, kind="ExternalInput")
    edge_embT = nc.dram_tensor("edge_embT", [F, T], FP, kind="ExternalInput")
    w_e = nc.dram_tensor("w_e", [F, H * F], FP, kind="ExternalInput")
    attn_e = nc.dram_tensor("attn_e", [T, H * F], FP, kind="ExternalInput")
    irp = nc.dram_tensor("irp", [NSP, H], BF, kind="ExternalOutput")
    eep = nc.dram_tensor("eep", [T, H], FP, kind="ExternalOutput")
    m1s = nc.dram_tensor("m1s", [P, ka * H], BF, kind="ExternalOutput")

    with tile.TileContext(nc) as tc:
        with (
            tc.tile_pool(name="sb", bufs=1) as sb,
            tc.tile_pool(name="ft", bufs=2) as ftp,
            tc.tile_pool(name="mm", bufs=3) as mm,
            tc.tile_pool(name="ps", bufs=2, space="PSUM") as ps,
        ):
            # --- wl/wr: contract W_fc[i, h*O+o] with attn_l/r[h, o] -> [i, 2H]
            wfc_t = [sb.tile([P, H * O], FP, tag=f"wfc{c}", name=f"wfc{c}") for c in range(2)]
            for c in range(2):
                nc.sync.dma_start(wfc_t[c][:], w_fc[c * P:(c + 1) * P, :])
            alr_t = sb.tile([P, 2 * H * O], FP)
            nc.sync.dma_start(alr_t[:], attn_lr[:])
            wlr = [sb.tile([P, 2 * H], BF, tag=f"wlr{c}", name=f"wlr{c}") for c in range(2)]
            for c in range(2):
                tmpw = sb.tile([P, 2 * H], FP, tag=f"wlf{c}", name=f"wlf{c}")
                for half in range(2):  # 0: attn_l, 1: attn_r
                    tmp = mm.tile([P, H * O], FP, tag="wtmp")
                    nc.vector.tensor_tensor(
                        tmp[:], wfc_t[c][:],
                        alr_t[:, half * H * O:(half + 1) * H * O],
                        mybir.AluOpType.mult)
                    nc.vector.tensor_reduce(
                        tmpw[:, half * H:(half + 1) * H],
                        tmp[:].rearrange("p (h o) -> p h o", h=H),
                        mybir.AxisListType.X, mybir.AluOpType.add)
                nc.vector.tensor_copy(wlr[c][:], tmpw[:])

            neg1 = sb.tile([P, 1], FP)
            nc.vector.memset(neg1[:], -1.0)

            # --- el'/ir' for the shard, slab-pipelined bf16 matmul.
            #     featT t-major: tile g reads columns [g*128, (g+1)*128).
            elb = sb.tile([P, G, H], BF)
            irb = sb.tile([P, G, H], BF)
            tt = 0
            while tt < G:
                nsl = min(SLICES, G - tt)
                ft = [ftp.tile([P, nsl * P], BF, tag=f"fts{c}", name=f"fts{c}")
                      for c in range(2)]
                for c in range(2):
                    nc.sync.dma_start(
                        ft[c][:], featT[c * P:(c + 1) * P, tt * P:(tt + nsl) * P])
                bank = ps.tile([P, SLICES * 2 * H], FP, tag="bank")
                for j in range(nsl):
                    sl = bank[:, j * 2 * H:(j + 1) * 2 * H]
                    for c in range(2):
                        nc.tensor.matmul(sl, lhsT=ft[c][:, j * P:(j + 1) * P],
                                         rhs=wlr[c][:],
                                         start=(c == 0), stop=(c == 1))
                bk = bank[:, :nsl * 2 * H].rearrange("p (t h) -> p t h", h=2 * H)
                nc.scalar.activation(elb[:, tt:tt + nsl, :], bk[:, :, 0:H],
                                     mybir.ActivationFunctionType.Exp)
                nc.scalar.activation(irb[:, tt:tt + nsl, :], bk[:, :, H:2 * H],
                                     mybir.ActivationFunctionType.Exp,
                                     scale=neg1[:])
                tt += nsl
            # ir' out: row g*128 + p holds the node at grid (p, g) (t-major,
            # same order as featT columns)
            nc.sync.dma_start(
                irp[:].rearrange("(t p) h -> p t h", p=P), irb[:])
            # --- ee table: (edge_emb @ W_e) [T, H*F] contract attn_e -> [T, H]
            embT_t = sb.tile([F, T], FP)
            nc.sync.dma_start(embT_t[:], edge_embT[:])
            we_t = sb.tile([F, H * F], FP)
            nc.sync.dma_start(we_t[:], w_e[:])
            ae_t = sb.tile([T, H * F], FP)
            nc.sync.dma_start(ae_t[:], attn_e[:])
            proj_ps = ps.tile([T, H * F], FP)
            nc.tensor.matmul(proj_ps[:], lhsT=embT_t[:], rhs=we_t[:],
                             start=True, stop=True)
            proj_sb = sb.tile([T, H * F], FP)
            nc.vector.tensor_tensor(
                proj_sb[:], proj_ps[:], ae_t[:],
                mybir.AluOpType.mult)
            ee_sb = sb.tile([T, H], FP)
            nc.vector.tensor_reduce(
                ee_sb[:], proj_sb[:].rearrange("t (h f) -> t h f", h=H),
                mybir.AxisListType.X, mybir.AluOpType.add)
            eep_sb = sb.tile([T, H], FP)
            nc.scalar.activation(eep_sb[:], ee_sb[:],
                                 mybir.ActivationFunctionType.Exp)
            nc.sync.dma_start(eep[:], eep_sb[:])
            # m1: replicate el'[p, g] across that node's out-edge slot columns
            for (g0, g1, wc, colbase) in chunks_a:
                cg = g1 - g0
                m1_t = mm.tile([P, cg, wc, H], BF, tag="m1")
                nc.vector.tensor_copy(
                    m1_t[:],
                    elb[:, g0:g1, :].unsqueeze(2).to_broadcast([P, cg, wc, H]))
                nc.scalar.dma_start(
                    m1s[:, colbase * H:(colbase + cg * wc) * H],
                    m1_t[:].rearrange("p c w h -> p (c w h)"))

    nc.compile()
    return nc


# ---------------------------------------------------------------------------
# Launch B: edge softmax over dst-grid slots (contiguous loads only)
# ---------------------------------------------------------------------------

def _build_launch_b(chunks_b, kb):
    nc = bacc.Bacc("TRN2", target_bir_lowering=False, debug=False,
                   num_devices=NCORES)
    ins = nc.dram_tensor("ins", [P, 2 * kb * H], BF, kind="ExternalInput")
    irg = nc.dram_tensor("irg", [P, G * H], BF, kind="ExternalInput")
    pads = nc.dram_tensor("pads", [P, G], FP, kind="ExternalInput")
    out = nc.dram_tensor("out", [P, kb * H], BF, kind="ExternalOutput")

    with tile.TileContext(nc) as tc:
        with (
            tc.tile_pool(name="cst", bufs=1) as cst,
            tc.tile_pool(name="inp", bufs=7) as inp,
            tc.tile_pool(name="sp", bufs=4) as sp,
        ):
            ir_sb = cst.tile([P, G, H], BF)
            nc.sync.dma_start(ir_sb[:],
                              irg[:].rearrange("p (g h) -> p g h", g=G))
            pads_sb = cst.tile([P, G], FP)
            nc.sync.dma_start(pads_sb[:], pads[:])
            # all pad corrections upfront (Pool, off the per-chunk chain):
            # corr[p, g, :] = -npad * ir
            corr_sb = cst.tile([P, G, H], FP)
            nc.gpsimd.tensor_tensor(
                corr_sb[:], ir_sb[:],
                pads_sb[:].unsqueeze(2).to_broadcast([P, G, H]),
                mybir.AluOpType.mult)

            DEPTH = 5   # chunks of load lookahead (manual software pipeline)

            def load(ck):
                g0, g1, wc, colbase = ck
                cg = g1 - g0
                lo, hi = colbase * H, (colbase + cg * wc) * H
                t2 = inp.tile([P, 2, cg, wc, H], BF, tag="t2")
                nc.sync.dma_start(
                    t2[:].rearrange("p s c w h -> p (s c w h)"),
                    ins[:, 2 * lo:2 * hi])
                return t2

            pending = [load(ck) for ck in chunks_b[:DEPTH]]
            for ci, (g0, g1, wc, colbase) in enumerate(chunks_b):
                cg = g1 - g0
                lo, hi = colbase * H, (colbase + cg * wc) * H
                t2 = pending[ci]
                if ci + DEPTH < len(chunks_b):
                    pending.append(load(chunks_b[ci + DEPTH]))
                el_v, ee_v = t2[:, 0], t2[:, 1]
                irb = ir_sb[:, g0:g1, :].unsqueeze(2).to_broadcast(
                    [P, cg, wc, H])
                # q = max(el*ee, ir)  (pad slots: el=ee=0 -> q = ir there,
                # corrected out of the segment sum below); DVE 2x fp16 (a Pool
                # mult here stalls the chain head -- Pool only takes the tail)
                nc.vector.tensor_tensor(ee_v, ee_v, el_v,
                                        mybir.AluOpType.mult)
                nc.vector.tensor_tensor(ee_v, ee_v, irb,
                                        mybir.AluOpType.max)
                # s = sum_w q: fp16 2x pair-add halves the 1x reduce read
                h_t = sp.tile([P, cg, wc // 2, H], BF, tag="half")
                qv = ee_v.rearrange("p c (w two) h -> p c w two h", two=2)
                nc.vector.tensor_tensor(
                    h_t[:], qv[:, :, :, 0, :], qv[:, :, :, 1, :],
                    mybir.AluOpType.add)
                s_t = sp.tile([P, cg, H], FP, tag="s")
                nc.vector.tensor_reduce(
                    s_t[:], h_t[:].rearrange("p c w h -> p c h w"),
                    mybir.AxisListType.X, mybir.AluOpType.add)
                # pad correction: s -= npad * ir (corr precomputed upfront)
                nc.gpsimd.tensor_tensor(s_t[:], s_t[:], corr_sb[:, g0:g1, :],
                                        mybir.AluOpType.subtract)
                r_t = sp.tile([P, cg, H], FP, tag="r")
                nc.vector.reciprocal(r_t[:].rearrange("p c h -> p (c h)"),
                                     s_t[:].rearrange("p c h -> p (c h)"))
                rb_t = sp.tile([P, cg, H], BF, tag="rb")
                nc.gpsimd.tensor_copy(
                    rb_t[:].rearrange("p c h -> p (c h)"),
                    r_t[:].rearrange("p c h -> p (c h)"))
                # out = q * r  (alternates DVE 2x / Pool to balance engines)
                feng = nc.vector if ci % 2 == 0 else nc.gpsimd
                feng.tensor_tensor(
                    el_v, ee_v,
                    rb_t[:].unsqueeze(2).to_broadcast([P, cg, wc, H]),
                    mybir.AluOpType.mult)
                nc.scalar.dma_start(
                    out[:, lo:hi],
                    el_v.rearrange("p c w h -> p (c w h)"))

    nc.compile()
    return nc


# ---------------------------------------------------------------------------
# Host orchestration
# ---------------------------------------------------------------------------

def _grid_structures(keys):
    """Per-core degree-sorted grids for one endpoint array (src or dst).

    Returns perm [NCORES, NSP] (sorted rank -> local node), cnt [NCORES, NSP],
    shared per-group width gw [G]."""
    perm = np.zeros((NCORES, NSP), np.int64)
    cnt = np.zeros((NCORES, NSP), np.int64)
    for c in range(NCORES):
        k = keys[(keys // NS) == c] - c * NS
        cc = np.bincount(k, minlength=NSP)
        pp = np.argsort(cc, kind="stable")
        perm[c] = pp
        cnt[c] = cc
    degs = np.take_along_axis(cnt, perm, axis=1)       # ascending per core
    gw = degs.reshape(NCORES, G, P).max(axis=2).max(axis=0)
    return perm, cnt, gw


def _slot_positions(keys, perm, cnt, chunks):
    """Per-edge slot coordinates on the degree-sorted grid of `keys`.

    Returns (core, p, col) arrays [E]."""
    core = keys // NS
    loc = keys - core * NS
    order = np.argsort(core * (2 * N) + loc, kind="stable")
    inv_perm = np.empty_like(perm)
    for c in range(NCORES):
        inv_perm[c, perm[c]] = np.arange(NSP)
    colbase_g = np.zeros(G, np.int64)
    for (g0, g1, wc, colbase) in chunks:
        for g in range(g0, g1):
            colbase_g[g] = colbase + (g - g0) * wc
    p_out = np.empty(E, np.int64)
    c_out = np.empty(E, np.int64)
    for c in range(NCORES):
        sel = order[core[order] == c]
        l = loc[sel]
        starts = np.concatenate([[0], np.cumsum(cnt[c])])
        rank = np.arange(len(sel)) - starts[l]
        r = inv_perm[c][l]
        p_out[sel] = r % P
        c_out[sel] = colbase_g[r // P] + rank
    return core, p_out, c_out


def kernel(feat, etype, src, dst, W_fc, edge_emb, W_e, attn_l, attn_r, attn_e):
    feat = np.asarray(feat)
    etype = np.asarray(etype).astype(np.int64)
    src = np.asarray(src).astype(np.int64)
    dst = np.asarray(dst).astype(np.int64)
    W_fc = np.asarray(W_fc)
    edge_emb = np.asarray(edge_emb)
    W_e = np.asarray(W_e)
    attn_l = np.asarray(attn_l)
    attn_r = np.asarray(attn_r)
    attn_e = np.asarray(attn_e)

    # ---------------- grid structure (integers only) ----------------
    perm_a, cnt_a, gw_a = _grid_structures(src)
    chunks_a, ka = _chunk_layout(gw_a)
    perm_b, cnt_b, gw_b = _grid_structures(dst)
    chunks_b, kb = _chunk_layout(gw_b)

    # ---------------- Launch A ----------------
    nc_a = _build_launch_a(chunks_a, ka)
    attn_lr = np.concatenate(
        [attn_l.reshape(1, H * O), attn_r.reshape(1, H * O)], axis=1)
    in_maps_a = []
    for s in range(NCORES):
        # featT t-major: column g*128 + p holds sorted-rank node perm_a[g*128+p]
        shard = np.zeros((NSP, IN), np.float32)
        shard[:NS] = feat[s * NS:(s + 1) * NS]
        featT_s = np.ascontiguousarray(
            shard[perm_a[s]].T.astype(np.float16))
        in_maps_a.append({
            "featT": featT_s,
            "w_fc": W_fc.astype(np.float32),
            "attn_lr": np.broadcast_to(attn_lr.astype(np.float32), (P, 2 * H * O)).copy(),
            "edge_embT": np.ascontiguousarray(edge_emb.T.astype(np.float32)),
            "w_e": W_e.astype(np.float32),
            "attn_e": np.broadcast_to(attn_e.reshape(1, H * F).astype(np.float32), (T, H * F)).copy(),
        })
    res_a = run_bass_kernel_spmd(nc_a, in_maps_a, core_ids=list(range(NCORES)))

    # device outputs (floats; host only permutes/replicates below)
    m1_all = np.stack([np.asarray(res_a.results[s]["m1s"]).reshape(P, ka, H)
                       for s in range(NCORES)])
    eep_host = np.asarray(res_a.results[0]["eep"])     # [T, H] f32
    # ir' by local node id, per core: irp row g*128+p = node perm_a[g*128+p]
    ir_node = np.zeros((NCORES, NSP, H), np.float16)
    for s in range(NCORES):
        ir_node[s, perm_a[s]] = np.asarray(res_a.results[s]["irp"])

    # ---------------- host bijection (indexing only) ----------------
    ca_core, ca_p, ca_col = _slot_positions(src, perm_a, cnt_a, chunks_a)
    cb_core, cb_p, cb_col = _slot_positions(dst, perm_b, cnt_b, chunks_b)

    el_vals = m1_all[ca_core, ca_p, ca_col]            # [E, H] bijection
    el_slot = np.zeros((NCORES, P, kb, H), np.float16)
    el_slot[cb_core, cb_p, cb_col] = el_vals
    ee_slot = np.zeros((NCORES, P, kb, H), np.float16)
    ee_slot[cb_core, cb_p, cb_col] = eep_host.astype(np.float16)[etype]
    slot_edge = np.full((NCORES, P, kb), -1, np.int64)
    slot_edge[cb_core, cb_p, cb_col] = np.arange(E)

    # ---------------- Launch B ----------------
    nc_b = _build_launch_b(chunks_b, kb)
    # pack [el | ee] per chunk so each chunk is one contiguous load
    elf = el_slot.reshape(NCORES, P, kb * H)
    eef = ee_slot.reshape(NCORES, P, kb * H)
    ins_pack = np.zeros((NCORES, P, 2 * kb * H), np.float16)
    wc_of_g = np.zeros(G, np.int64)
    for (g0, g1, wc, colbase) in chunks_b:
        wc_of_g[g0:g1] = wc
        lo = colbase * H
        w = (g1 - g0) * wc * H
        ins_pack[:, :, 2 * lo:2 * lo + w] = elf[:, :, lo:lo + w]
        ins_pack[:, :, 2 * lo + w:2 * lo + 2 * w] = eef[:, :, lo:lo + w]
    in_maps_b = []
    for c in range(NCORES):
        nodes_pg = perm_b[c].reshape(G, P)             # (g, p) -> local node
        ir_grid = ir_node[c][nodes_pg].transpose(1, 0, 2)   # [P, G, H]
        pads_np = (wc_of_g[None, :]
                   - np.maximum(cnt_b[c][nodes_pg], 1).T).astype(np.float32)
        in_maps_b.append({
            "ins": ins_pack[c],
            "irg": np.ascontiguousarray(ir_grid.reshape(P, G * H)),
            "pads": np.ascontiguousarray(pads_np),
        })
    res_b = run_bass_kernel_spmd(nc_b, in_maps_b, core_ids=list(range(NCORES)))

    # ---------------- unshard ----------------
    out = np.zeros((E, H), np.float32)
    for c in range(NCORES):
        o_c = np.asarray(res_b.results[c]["out"]).reshape(P, kb, H)
        mask = slot_edge[c] >= 0
        out[slot_edge[c][mask]] = o_c[mask].astype(np.float32)

    # timing estimate via the cost-model simulator (no NTFF profiling
    # available under this axon client; see test.py)
    try:
        from concourse.timeline_sim import TimelineSim
        _timings["A_ns"] = TimelineSim(nc_a).simulate()
        _timings["B_ns"] = TimelineSim(nc_b).simulate()
    except Exception as ex:  # timing must never break correctness
        _timings["error"] = repr(ex)

    return out


# revision 64
# speedup vs baseline: 1.0029x; 1.0029x over previous
"""Trainium2 Bass kernel for nn_AttentionWeight (GAT edge softmax).

out[e,h] = softmax_over_dst_segments(relu(el[src]+er[dst]+ee[etype]))

Math used on device (er cancels out of the softmax):
  exp(relu(x)) = max(exp(x), 1),  exp(x) = el'*ee'*er'   (primes = exp factors)
  max(el'*ee'*er', 1) = er' * max(el'*ee', 1/er') = er' * max(m2, ir)
  out = max(m2, ir) / segment_sum(max(m2, ir))           (er' cancels; ir=1/er')
  Padding slots carry m2 = 0 and ir = 0, so q = max(m2, ir) = 0 there and the
  segment sum needs no degree correction.

Distribution (8 NeuronCores, two SPMD launches):
  Launch A: node-sharded projections (fp16 feat, fp16 matmul). Core s owns
    nodes [12500s, 12500(s+1)) on an out-degree-sorted grid [128 x 98] (the
    sort is a host-side column permutation of featT):
      el' = exp(+logit_l), ir' = exp(-logit_r)  (ACT, fp16 out)
      ee' = exp(contract(edge_emb@W_e, attn_e))       [8 x 8]
    The per-edge el' values are emitted on device: for each node group, its
    el' row is replicated across that node's out-edge slot columns (DVE 4x
    fp16 broadcast copy), so every edge's el' factor leaves the device in
    src-grid slot order (m1s).
  Host: bijectively re-shards the per-edge el' slot values from src-grid to
    dst-grid order (each edge's 8 floats appear exactly once on each side),
    and expands the tiny device-computed ee' [8x8] / ir' [nodes x 8] tables
    into per-slot marshalled inputs. Integer index arrays + pure permutation /
    replication of device-produced floats only -- no float arithmetic.
  Launch B: edge/dst-sharded softmax. Core c owns dst range on an in-degree-
    sorted grid, edges padded into chunk-uniform slot bands (~3-5% padding),
    inputs packed [el | ee | ir] per chunk so each chunk is one contiguous
    load, software-pipelined 4 chunks ahead. Per chunk (fp16, DVE 2x):
    m2 = el*ee; q = max(m2, ir); segment sums via fp16 pair-add + strided
    X-reduce (f32 accumulate); r = 1/s; out = q*r. Contiguous DMA only --
    no indirect gathers.
  Host: scatters padded slots back to original edge order (indexing only).

All floating-point arithmetic happens on device; the host only shards,
permutes, concatenates and builds integer index/count arrays.
"""

import sys

sys.path.insert(0, "/opt/trn_rl_repo")

import numpy as np

import concourse.bacc as bacc
import concourse.mybir as mybir
import concourse.tile as tile
from concourse.bass_utils import run_bass_kernel_spmd

# problem constants (hardcoded per harness contract)
N = 100000
E = 3200000
IN = 256
H = 8
O = 64
F = 64
T = 8
NCORES = 8
P = 128

NS = N // NCORES            # 12500 nodes per shard
NSP = 12544                 # padded to 128*98
G = NSP // P                # 98 groups of 128 nodes

FP = mybir.dt.float32
BF = mybir.dt.float16
I32 = mybir.dt.int32

CH = 8                      # groups per chunk (uniform slot width per chunk)
SLICES = 32                 # node groups per PSUM bank / featT slab

_timings = {}


def _chunk_layout(gw):
    """gw: per-group slot width [G]. Returns list of (g0, g1, wc, colbase)
    with uniform width wc = max(gw[g0:g1]) per chunk, and total columns."""
    chunks = []
    colbase = 0
    g0 = 0
    while g0 < G:
        g1 = min(g0 + CH, G)
        wc = int(max(2, max(gw[g0:g1])))
        wc += wc & 1                       # even width (pair-add reduce)
        chunks.append((g0, g1, wc, colbase))
        colbase += (g1 - g0) * wc
        g0 = g1
    return chunks, colbase


# ---------------------------------------------------------------------------
# Launch A: projections + per-edge el' emission (src grid)
# ---------------------------------------------------------------------------

def _build_launch_a(chunks_a, ka):
    nc = bacc.Bacc("TRN2", target_bir_lowering=False, debug=False,
                   num_devices=NCORES)
    # featT columns are t-major: column g*128 + p holds (sorted) grid node (p, g)
    featT = nc.dram_tensor("featT", [IN, NSP], BF, kind="ExternalInput")
    w_fc = nc.dram_tensor("w_fc", [IN, H * O], FP, kind="ExternalInput")
    attn_lr = nc.dram_tensor("attn_lr", [P, 2 * H * O], FP, kind="ExternalInput")
    edge_embT = nc.dram_tensor("edge_embT", [F, T], FP, kind="ExternalInput")
    w_e = nc.dram_tensor("w_e", [F, H * F], FP, kind="ExternalInput")
    attn_e = nc.dram_tensor("attn_e", [T, H * F], FP, kind="ExternalInput")
    irp = nc.dram_tensor("irp", [NSP, H], BF, kind="ExternalOutput")
    eep = nc.dram_tensor("eep", [T, H], FP, kind="ExternalOutput")
    m1s = nc.dram_tensor("m1s", [P, ka * H], BF, kind="ExternalOutput")

    with tile.TileContext(nc) as tc:
        with (
            tc.tile_pool(name="sb", bufs=1) as sb,
            tc.tile_pool(name="ft", bufs=2) as ftp,
            tc.tile_pool(name="mm", bufs=3) as mm,
            tc.tile_pool(name="ps", bufs=2, space="PSUM") as ps,
        ):
            # --- wl/wr: contract W_fc[i, h*O+o] with attn_l/r[h, o] -> [i, 2H]
            wfc_t = [sb.tile([P, H * O], FP, tag=f"wfc{c}", name=f"wfc{c}") for c in range(2)]
            for c in range(2):
                nc.sync.dma_start(wfc_t[c][:], w_fc[c * P:(c + 1) * P, :])
            alr_t = sb.tile([P, 2 * H * O], FP)
            nc.sync.dma_start(alr_t[:], attn_lr[:])
            wlr = [sb.tile([P, 2 * H], BF, tag=f"wlr{c}", name=f"wlr{c}") for c in range(2)]
            for c in range(2):
                tmpw = sb.tile([P, 2 * H], FP, tag=f"wlf{c}", name=f"wlf{c}")
                for half in range(2):  # 0: attn_l, 1: attn_r
                    tmp = mm.tile([P, H * O], FP, tag="wtmp")
                    nc.vector.tensor_tensor(
                        tmp[:], wfc_t[c][:],
                        alr_t[:, half * H * O:(half + 1) * H * O],
                        mybir.AluOpType.mult)
                    nc.vector.tensor_reduce(
                        tmpw[:, half * H:(half + 1) * H],
                        tmp[:].rearrange("p (h o) -> p h o", h=H),
                        mybir.AxisListType.X, mybir.AluOpType.add)
                nc.vector.tensor_copy(wlr[c][:], tmpw[:])

            # --- ee table: (edge_emb @ W_e) [T, H*F] contract attn_e -> [T, H]
            embT_t = sb.tile([F, T], FP)
            nc.sync.dma_start(embT_t[:], edge_embT[:])
            we_t = sb.tile([F, H * F], FP)
            nc.sync.dma_start(we_t[:], w_e[:])
            ae_t = sb.tile([T, H * F], FP)
            nc.sync.dma_start(ae_t[:], attn_e[:])
            proj_ps = ps.tile([T, H * F], FP)
            nc.tensor.matmul(proj_ps[:], lhsT=embT_t[:], rhs=we_t[:],
                             start=True, stop=True)
            proj_sb = sb.tile([T, H * F], FP)
            nc.vector.tensor_tensor(
                proj_sb[:], proj_ps[:], ae_t[:],
                mybir.AluOpType.mult)
            ee_sb = sb.tile([T, H], FP)
            nc.vector.tensor_reduce(
                ee_sb[:], proj_sb[:].rearrange("t (h f) -> t h f", h=H),
                mybir.AxisListType.X, mybir.AluOpType.add)
            eep_sb = sb.tile([T, H], FP)
            nc.scalar.activation(eep_sb[:], ee_sb[:],
                                 mybir.ActivationFunctionType.Exp)
            nc.sync.dma_start(eep[:], eep_sb[:])

            neg1 = sb.tile([P, 1], FP)
            nc.vector.memset(neg1[:], -1.0)

            # --- el'/ir' for the shard, slab-pipelined bf16 matmul.
            #     featT t-major: tile g reads columns [g*128, (g+1)*128).
            elb = sb.tile([P, G, H], BF)
            irb = sb.tile([P, G, H], BF)
            tt = 0
            while tt < G:
                nsl = min(SLICES, G - tt)
                ft = [ftp.tile([P, nsl * P], BF, tag=f"fts{c}", name=f"fts{c}")
                      for c in range(2)]
                for c in range(2):
                    nc.sync.dma_start(
                        ft[c][:], featT[c * P:(c + 1) * P, tt * P:(tt + nsl) * P])
                bank = ps.tile([P, SLICES * 2 * H], FP, tag="bank")
                for j in range(nsl):
                    sl = bank[:, j * 2 * H:(j + 1) * 2 * H]
                    for c in range(2):
                        nc.tensor.matmul(sl, lhsT=ft[c][:, j * P:(j + 1) * P],
                                         rhs=wlr[c][:],
                                         start=(c == 0), stop=(c == 1))
                bk = bank[:, :nsl * 2 * H].rearrange("p (t h) -> p t h", h=2 * H)
                nc.scalar.activation(elb[:, tt:tt + nsl, :], bk[:, :, 0:H],
                                     mybir.ActivationFunctionType.Exp)
                nc.scalar.activation(irb[:, tt:tt + nsl, :], bk[:, :, H:2 * H],
                                     mybir.ActivationFunctionType.Exp,
                                     scale=neg1[:])
                tt += nsl
            # ir' out: row g*128 + p holds the node at grid (p, g) (t-major,
            # same order as featT columns)
            nc.sync.dma_start(
                irp[:].rearrange("(t p) h -> p t h", p=P), irb[:])
            # m1: replicate el'[p, g] across that node's out-edge slot columns
            for (g0, g1, wc, colbase) in chunks_a:
                cg = g1 - g0
                m1_t = mm.tile([P, cg, wc, H], BF, tag="m1")
                nc.vector.tensor_copy(
                    m1_t[:],
                    elb[:, g0:g1, :].unsqueeze(2).to_broadcast([P, cg, wc, H]))
                nc.scalar.dma_start(
                    m1s[:, colbase * H:(colbase + cg * wc) * H],
                    m1_t[:].rearrange("p c w h -> p (c w h)"))

    nc.compile()
    return nc


# ---------------------------------------------------------------------------
# Launch B: edge softmax over dst-grid slots (contiguous loads only)
# ---------------------------------------------------------------------------

def _build_launch_b(chunks_b, kb):
    nc = bacc.Bacc("TRN2", target_bir_lowering=False, debug=False,
                   num_devices=NCORES)
    ins = nc.dram_tensor("ins", [P, 2 * kb * H], BF, kind="ExternalInput")
    irg = nc.dram_tensor("irg", [P, G * H], BF, kind="ExternalInput")
    pads = nc.dram_tensor("pads", [P, G], FP, kind="ExternalInput")
    out = nc.dram_tensor("out", [P, kb * H], BF, kind="ExternalOutput")

    with tile.TileContext(nc) as tc:
        with (
            tc.tile_pool(name="cst", bufs=1) as cst,
            tc.tile_pool(name="inp", bufs=7) as inp,
            tc.tile_pool(name="sp", bufs=4) as sp,
        ):
            ir_sb = cst.tile([P, G, H], BF)
            nc.sync.dma_start(ir_sb[:],
                              irg[:].rearrange("p (g h) -> p g h", g=G))
            pads_sb = cst.tile([P, G], FP)
            nc.sync.dma_start(pads_sb[:], pads[:])
            # all pad corrections upfront (Pool, off the per-chunk chain):
            # corr[p, g, :] = -npad * ir
            corr_sb = cst.tile([P, G, H], FP)
            nc.gpsimd.tensor_tensor(
                corr_sb[:], ir_sb[:],
                pads_sb[:].unsqueeze(2).to_broadcast([P, G, H]),
                mybir.AluOpType.mult)

            DEPTH = 5   # chunks of load lookahead (manual software pipeline)

            def load(ck):
                g0, g1, wc, colbase = ck
                cg = g1 - g0
                lo, hi = colbase * H, (colbase + cg * wc) * H
                t2 = inp.tile([P, 2, cg, wc, H], BF, tag="t2")
                nc.sync.dma_start(
                    t2[:].rearrange("p s c w h -> p (s c w h)"),
                    ins[:, 2 * lo:2 * hi])
                return t2

            pending = [load(ck) for ck in chunks_b[:DEPTH]]
            for ci, (g0, g1, wc, colbase) in enumerate(chunks_b):
                cg = g1 - g0
                lo, hi = colbase * H, (colbase + cg * wc) * H
                t2 = pending[ci]
                if ci + DEPTH < len(chunks_b):
                    pending.append(load(chunks_b[ci + DEPTH]))
                el_v, ee_v = t2[:, 0], t2[:, 1]
                irb = ir_sb[:, g0:g1, :].unsqueeze(2).to_broadcast(
                    [P, cg, wc, H])
                # q = max(el*ee, ir)  (pad slots: el=ee=0 -> q = ir there,
                # corrected out of the segment sum below); DVE 2x fp16 (a Pool
                # mult here stalls the chain head -- Pool only takes the tail)
                nc.vector.tensor_tensor(ee_v, ee_v, el_v,
                                        mybir.AluOpType.mult)
                nc.vector.tensor_tensor(ee_v, ee_v, irb,
                                        mybir.AluOpType.max)
                # s = sum_w q: fp16 2x pair-add halves the 1x reduce read
                h_t = sp.tile([P, cg, wc // 2, H], BF, tag="half")
                qv = ee_v.rearrange("p c (w two) h -> p c w two h", two=2)
                nc.vector.tensor_tensor(
                    h_t[:], qv[:, :, :, 0, :], qv[:, :, :, 1, :],
                    mybir.AluOpType.add)
                s_t = sp.tile([P, cg, H], FP, tag="s")
                nc.vector.tensor_reduce(
                    s_t[:], h_t[:].rearrange("p c w h -> p c h w"),
                    mybir.AxisListType.X, mybir.AluOpType.add)
                # pad correction: s -= npad * ir (corr precomputed upfront)
                nc.gpsimd.tensor_tensor(s_t[:], s_t[:], corr_sb[:, g0:g1, :],
                                        mybir.AluOpType.subtract)
                r_t = sp.tile([P, cg, H], FP, tag="r")
                nc.vector.reciprocal(r_t[:].rearrange("p c h -> p (c h)"),
                                     s_t[:].rearrange("p c h -> p (c h)"))
                rb_t = sp.tile([P, cg, H], BF, tag="rb")
                nc.gpsimd.tensor_copy(
                    rb_t[:].rearrange("p c h -> p (c h)"),
                    r_t[:].rearrange("p c h -> p (c h)"))
                # out = q * r  (alternates DVE 2x / Pool to balance engines)
                feng = nc.vector if ci % 2 == 0 else nc.gpsimd
                feng.tensor_tensor(
                    el_v, ee_v,
                    rb_t[:].unsqueeze(2).to_broadcast([P, cg, wc, H]),
                    mybir.AluOpType.mult)
                nc.scalar.dma_start(
                    out[:, lo:hi],
                    el_v.rearrange("p c w h -> p (c w h)"))

    nc.compile()
    return nc


# ---------------------------------------------------------------------------
# Host orchestration
# ---------------------------------------------------------------------------

def _grid_structures(keys):
    """Per-core degree-sorted grids for one endpoint array (src or dst).

    Returns perm [NCORES, NSP] (sorted rank -> local node), cnt [NCORES, NSP],
    shared per-group width gw [G]."""
    perm = np.zeros((NCORES, NSP), np.int64)
    cnt = np.zeros((NCORES, NSP), np.int64)
    for c in range(NCORES):
        k = keys[(keys // NS) == c] - c * NS
        cc = np.bincount(k, minlength=NSP)
        pp = np.argsort(cc, kind="stable")
        perm[c] = pp
        cnt[c] = cc
    degs = np.take_along_axis(cnt, perm, axis=1)       # ascending per core
    gw = degs.reshape(NCORES, G, P).max(axis=2).max(axis=0)
    return perm, cnt, gw


def _slot_positions(keys, perm, cnt, chunks):
    """Per-edge slot coordinates on the degree-sorted grid of `keys`.

    Returns (core, p, col) arrays [E]."""
    core = keys // NS
    loc = keys - core * NS
    order = np.argsort(core * (2 * N) + loc, kind="stable")
    inv_perm = np.empty_like(perm)
    for c in range(NCORES):
        inv_perm[c, perm[c]] = np.arange(NSP)
    colbase_g = np.zeros(G, np.int64)
    for (g0, g1, wc, colbase) in chunks:
        for g in range(g0, g1):
            colbase_g[g] = colbase + (g - g0) * wc
    p_out = np.empty(E, np.int64)
    c_out = np.empty(E, np.int64)
    for c in range(NCORES):
        sel = order[core[order] == c]
        l = loc[sel]
        starts = np.concatenate([[0], np.cumsum(cnt[c])])
        rank = np.arange(len(sel)) - starts[l]
        r = inv_perm[c][l]
        p_out[sel] = r % P
        c_out[sel] = colbase_g[r // P] + rank
    return core, p_out, c_out


def kernel(feat, etype, src, dst, W_fc, edge_emb, W_e, attn_l, attn_r, attn_e):
    feat = np.asarray(feat)
    etype = np.asarray(etype).astype(np.int64)
    src = np.asarray(src).astype(np.int64)
    dst = np.asarray(dst).astype(np.int64)
    W_fc = np.asarray(W_fc)
    edge_emb = np.asarray(edge_emb)
    W_e = np.asarray(W_e)
    attn_l = np.asarray(attn_l)
    attn_r = np.asarray(attn_r)
    attn_e = np.asarray(attn_e)

    # ---------------- grid structure (integers only) ----------------
    perm_a, cnt_a, gw_a = _grid_structures(src)
    chunks_a, ka = _chunk_layout(gw_a)
    perm_b, cnt_b, gw_b = _grid_structures(dst)
    chunks_b, kb = _chunk_layout(gw_b)

    # ---------------- Launch A ----------------
    nc_a = _build_launch_a(chunks_a, ka)
    attn_lr = np.concatenate(
        [attn_l.reshape(1, H * O), attn_r.reshape(1, H * O)], axis=1)
    in_maps_a = []
    for s in range(NCORES):
        # featT t-major: column g*128 + p holds sorted-rank node perm_a[g*128+p]
        shard = np.zeros((NSP, IN), np.float32)
        shard[:NS] = feat[s * NS:(s + 1) * NS]
        featT_s = np.ascontiguousarray(
            shard[perm_a[s]].T.astype(np.float16))
        in_maps_a.append({
            "featT": featT_s,
            "w_fc": W_fc.astype(np.float32),
            "attn_lr": np.broadcast_to(attn_lr.astype(np.float32), (P, 2 * H * O)).copy(),
            "edge_embT": np.ascontiguousarray(edge_emb.T.astype(np.float32)),
            "w_e": W_e.astype(np.float32),
            "attn_e": np.broadcast_to(attn_e.reshape(1, H * F).astype(np.float32), (T, H * F)).copy(),
        })
    res_a = run_bass_kernel_spmd(nc_a, in_maps_a, core_ids=list(range(NCORES)))

    # device outputs (floats; host only permutes/replicates below)
    m1_all = np.stack([np.asarray(res_a.results[s]["m1s"]).reshape(P, ka, H)
                       for s in range(NCORES)])
    eep_host = np.asarray(res_a.results[0]["eep"])     # [T, H] f32
    # ir' by local node id, per core: irp row g*128+p = node perm_a[g*128+p]
    ir_node = np.zeros((NCORES, NSP, H), np.float16)
    for s in range(NCORES):
        ir_node[s, perm_a[s]] = np.asarray(res_a.results[s]["irp"])

    # ---------------- host bijection (indexing only) ----------------
    ca_core, ca_p, ca_col = _slot_positions(src, perm_a, cnt_a, chunks_a)
    cb_core, cb_p, cb_col = _slot_positions(dst, perm_b, cnt_b, chunks_b)

    el_vals = m1_all[ca_core, ca_p, ca_col]            # [E, H] bijection
    el_slot = np.zeros((NCORES, P, kb, H), np.float16)
    el_slot[cb_core, cb_p, cb_col] = el_vals
    ee_slot = np.zeros((NCORES, P, kb, H), np.float16)
    ee_slot[cb_core, cb_p, cb_col] = eep_host.astype(np.float16)[etype]
    slot_edge = np.full((NCORES, P, kb), -1, np.int64)
    slot_edge[cb_core, cb_p, cb_col] = np.arange(E)

    # ---------------- Launch B ----------------
    nc_b = _build_launch_b(chunks_b, kb)
    # pack [el | ee] per chunk so each chunk is one contiguous load
    elf = el_slot.reshape(NCORES, P, kb * H)
    eef = ee_slot.reshape(NCORES, P, kb * H)
    ins_pack = np.zeros((NCORES, P, 2 * kb * H), np.float16)
    wc_of_g = np.zeros(G, np.int64)
    for (g0, g1, wc, colbase) in chunks_b:
        wc_of_g[g0:g1] = wc
        lo = colbase * H
        w = (g1 - g0) * wc * H
        ins_pack[:, :, 2 * lo:2 * lo + w] = elf[:, :, lo:lo + w]
        ins_pack[:, :, 2 * lo + w:2 * lo + 2 * w] = eef[:, :, lo:lo + w]
    in_maps_b = []
    for c in range(NCORES):
        nodes_pg = perm_b[c].reshape(G, P)             # (g, p) -> local node
        ir_grid = ir_node[c][nodes_pg].transpose(1, 0, 2)   # [P, G, H]
        pads_np = (wc_of_g[None, :]
                   - np.maximum(cnt_b[c][nodes_pg], 1).T).astype(np.float32)
        in_maps_b.append({
            "ins": ins_pack[c],
            "irg": np.ascontiguousarray(ir_grid.reshape(P, G * H)),
            "pads": np.ascontiguousarray(pads_np),
        })
    res_b = run_bass_kernel_spmd(nc_b, in_maps_b, core_ids=list(range(NCORES)))

    # ---------------- unshard ----------------
    out = np.zeros((E, H), np.float32)
    for c in range(NCORES):
        o_c = np.asarray(res_b.results[c]["out"]).reshape(P, kb, H)
        mask = slot_edge[c] >= 0
        out[slot_edge[c][mask]] = o_c[mask].astype(np.float32)

    # timing estimate via the cost-model simulator (no NTFF profiling
    # available under this axon client; see test.py)
    try:
        from concourse.timeline_sim import TimelineSim
        _timings["A_ns"] = TimelineSim(nc_a).simulate()
        _timings["B_ns"] = TimelineSim(nc_b).simulate()
    except Exception as ex:  # timing must never break correctness
        _timings["error"] = repr(ex)

    return out
